# revision 34
# baseline (speedup 1.0000x reference)
"""Trainium2 Bass kernel for nn_MessageLayer (GNN message passing), 8 NeuronCores.

Reference computation:
    edge_mat = (edge_features @ W + b).reshape(E, 64, 16)
    messages = einsum('emh,eh->em', edge_mat, hidden[edge_sources])
    out      = segment_sum(messages, edge_targets, num_segments=10000)

Algebraic restructure (cuts FLOPs 32x): since aggregation is linear,
    out[n, m] = sum_{f,h} W[f, m*16+h] * C[n, f, h],
    C[n, f, h] = sum_{e: tgt(e)=n} ef[e, f] * hidden[src(e), h]
Then out = C @ Wr as 32 accumulating matmuls against a block-diagonal W.

C-stage structure (v5): the PE issue rate (~35 ns per matmul call, nearly
independent of operand size) dominates, so segments are batched FOUR per
matmul.  Segments are split to <=32 edges and packed four to a 128-row tile
(group g = rows 32g..32g+32).  Per tile ONE matmul:
    lhsT = ef   [rows, 32]   (compact edge features, all four groups)
    rhs  = nhbd [rows, 64]   (block-diagonal: group g's source-hidden in
                              cols 16g..16g+16, zeros elsewhere)
    out  = [32f, 64=(g,h)] in PSUM at partition group 32q, q = tile%4
so C for four segments lands in one PSUM write.  All matmuls contract from
row 0 (same PE row-group) so they serialize — no concurrent PSUM-bank
drains (the known wedge).  The block-diagonal rhs is packed on the host
(zeros ship from HBM; ~2x input bytes, still cheaper than 3x the matmul
calls).

All matmul operands are bf16 (PSUM stays fp32): 1 cycle/row vs fp32's 4,
half the DMA.  Quantization error ~0.3% vs the 2e-2 gate.

Sharding: node-ownership, no collective.  Nodes are dealt snake-wise in
descending-degree order so per-core sorted segment profiles match and the
SPMD cross-core max-padding (K_j) is minimal.

c_all is h-major so the W-stage moving operand c_all[:, h, :] is contiguous
(a strided moving was measured 3x slower).  Every input chunk is split
across both HW DGE queues (SP + Activation) so each chunk completes at full
aggregate HBM rate.
"""
import numpy as np
import ml_dtypes
from contextlib import ExitStack

BF16 = ml_dtypes.bfloat16

N_NODES = 10000
N_EDGES = 320000
HID = 16
MSG = 64
EFD = 32
NCORES = 8
RW = EFD + 4 * HID               # 96 packed cols per row: ef | nhbd(4x16)
# progressive input chunks: small first chunk so the PE starts early, taper
# at the end so the last casts (which gate the W stage) come early
CHUNK_FRACS = (0.0, 0.015, 0.05, 0.10, 0.16, 0.23, 0.31, 0.40, 0.49, 0.58,
               0.67, 0.76, 0.85, 0.93, 1.0)
NCHUNK = len(CHUNK_FRACS) - 1
WBD_AFTER = 4                    # ship wbd after this chunk (needed by W-A)
NBA = 8                          # banks 0..NBA-1 go to the early W-A pass

_CACHE = {}


def _build_layout(edge_targets):
    """Per-core segment lists (<=32 edges each, K-sorted) + SPMD-uniform K."""
    deg = np.bincount(edge_targets, minlength=N_NODES)
    order = np.argsort(-deg, kind="stable")      # nodes by degree desc
    node_core = np.empty(N_NODES, dtype=np.int64)
    snake = list(range(NCORES)) + list(range(NCORES - 1, -1, -1))
    for i, n in enumerate(order):
        node_core[n] = snake[i % (2 * NCORES)]

    order_e = np.argsort(edge_targets, kind="stable")
    tgt_sorted = edge_targets[order_e]
    uniq, starts = np.unique(tgt_sorted, return_index=True)
    bounds = list(starts) + [len(tgt_sorted)]

    segs_per_core = [[] for _ in range(NCORES)]
    for i, n in enumerate(uniq):
        s, e = bounds[i], bounds[i + 1]
        c = node_core[n]
        while e - s > 32:           # split to <=32; host re-adds partials
            segs_per_core[c].append((int(n), order_e[s:s + 32]))
            s += 32
        segs_per_core[c].append((int(n), order_e[s:e]))
    for c in range(NCORES):
        segs_per_core[c].sort(key=lambda t: -len(t[1]))

    NPOS = max(len(s) for s in segs_per_core)
    NPOS = ((NPOS + 3) // 4) * 4
    K = np.ones(NPOS, dtype=np.int64)
    for segs in segs_per_core:
        for j, (_, e) in enumerate(segs):
            K[j] = max(K[j], len(e))
    T = NPOS // 4
    NB = (T + 31) // 32
    return segs_per_core, NPOS, K, T, NB


def _pack_core(segs, NPOS, K, T, wbd, edge_features, edge_sources, hidden):
    # position j -> tile t=j//4, group g=j%4, rows 32g..32g+K_j of tile t
    ef = np.zeros((T * 128, EFD), dtype=np.float32)
    nh = np.zeros((T * 128, HID), dtype=np.float32)
    for j in range(min(len(segs), NPOS)):
        _, eids = segs[j]
        base = (j // 4) * 128 + 32 * (j % 4)
        ef[base:base + len(eids)] = edge_features[eids]
        nh[base:base + len(eids)] = hidden[edge_sources[eids]]
    d = np.zeros((T * 128, RW), dtype=np.float32)
    d[:, :EFD] = ef
    g_of_row = (np.arange(T * 128) % 128) // 32
    for g in range(4):
        m = g_of_row == g
        d[m, EFD + HID * g:EFD + HID * (g + 1)] = nh[m]
    # DRAM [128 partitions, T*96 + wbd]: tile t at free offset 96t
    d = d.reshape(T, 128, RW).swapaxes(0, 1).reshape(128, T * RW)
    return np.ascontiguousarray(np.concatenate([d.astype(BF16), wbd], axis=1))


def _build_wrep(W):
    # Wrep[(q,f), 32p + m] = W[f, (32*half+m)*16 + h], p = 2h+half;
    # on-chip [32,32] copies place block q of phase p from this compact form
    Wr = W.reshape(EFD, MSG, HID)                      # [f, m, h]
    wrep = np.empty((EFD, 32, 32), dtype=np.float32)   # [f, p, m]
    for h in range(HID):
        for half in range(2):
            wrep[:, 2 * h + half, :] = Wr[:, 32 * half:32 * half + 32, h]
    flat = wrep.reshape(EFD, 32 * 32)
    return np.ascontiguousarray(np.tile(flat, (4, 1))).astype(BF16)


def _chunk_bounds(T):
    b = [round(f * T) for f in CHUNK_FRACS]
    b[-1] = T
    return b


def _build_program(NPOS, K, T, NB):
    import concourse.tile as tile
    from concourse import bacc, mybir

    f32 = mybir.dt.float32
    bf16 = mybir.dt.bfloat16
    bounds = _chunk_bounds(T)

    nc = bacc.Bacc("TRN2", target_bir_lowering=False, debug=False,
                   num_devices=NCORES)
    data_dram = nc.dram_tensor("data", [128, T * RW + 1024], bf16,
                               kind="ExternalInput").ap()
    out_dram = nc.dram_tensor("out", [128, 2 * NB * 32], f32,
                              kind="ExternalOutput").ap()

    with tile.TileContext(nc) as tc, ExitStack() as ctx:
        big = ctx.enter_context(tc.tile_pool(name="big", bufs=1))
        cpool = ctx.enter_context(tc.tile_pool(name="cps", bufs=3,
                                               space="PSUM"))
        opool = ctx.enter_context(tc.tile_pool(name="ops", bufs=2, space="PSUM"))

        # on-chip wbd: ship compact Wrep (256 KB), zero the 1 MB block-diag
        # form once, then 128 [32,32] engine copies place the blocks --
        # all during the DMA-bound phase
        wrep_sb = big.tile([128, 1024], bf16, tag="wrep", name="wrep")
        nc.scalar.dma_start(wrep_sb[:], data_dram[:, T * RW:])
        wbd_sb = big.tile([128, 32 * 128], bf16, tag="wbd", name="wbd")
        nc.vector.memset(wbd_sb[:, :2048], 0.0)
        nc.gpsimd.memset(wbd_sb[:, 2048:], 0.0)

        ch_sb = []
        for k in range(NCHUNK):
            lo, hi = bounds[k] * RW, bounds[k + 1] * RW
            t = big.tile([128, hi - lo], bf16, tag=f"ch{k}", name=f"ch{k}")
            # full-width transfers, alternating HW DGE queues (a [64,:]
            # partition split measured only ~275 GB/s aggregate vs ~405)
            eng = nc.sync if k % 2 == 0 else nc.scalar
            eng.dma_start(t[:], data_dram[:, lo:hi])
            ch_sb.append(t)

        for p in range(32):          # place wbd blocks from compact Wrep
            for q in range(4):
                src = wrep_sb[32 * q:32 * q + 32, 32 * p:32 * p + 32]
                dst = wbd_sb[32 * q:32 * q + 32,
                             128 * p + 32 * q:128 * p + 32 * q + 32]
                if (p + q) % 2 == 0:
                    nc.vector.tensor_copy(dst, src)
                else:
                    nc.gpsimd.tensor_copy(dst, src)

        # h-major: c_all[p, h, 32*b + w2], w2 = 4*j8 + g.  Columns of the
        # partial last bank with no backing tile carry PSUM garbage; the
        # W-stage is column-independent and _assemble never reads them.
        c_all = big.tile([128, HID, NB * 32], bf16, tag="call")

        out_sb = big.tile([128, 2 * NB * 32], f32, tag="outsb")

        def w_stage(lo_b, hi_b):
            # out = C @ Wr restricted to banks [lo_b, hi_b)
            lo, hi = 32 * lo_b, 32 * hi_b
            for half in range(2):
                po = opool.tile([128, NBA * 32], f32, tag="po",
                                name=f"po{lo_b}_{half}")[:, :hi - lo]
                for h in range(HID):
                    p = 2 * h + half
                    nc.tensor.matmul(
                        po[:], wbd_sb[:, 128 * p:128 * p + 128],
                        c_all[:, h, lo:hi],
                        start=(h == 0), stop=(h == HID - 1))
                sl = slice(NB * 32 * half + lo, NB * 32 * half + hi)
                if half == 0:
                    nc.vector.tensor_copy(out_sb[:, sl], po[:])
                else:
                    nc.scalar.copy(out_sb[:, sl], po[:])
                (nc.sync if half == 0 else nc.scalar).dma_start(
                    out_dram[:, sl], out_sb[:, sl])

        chunk_of = np.searchsorted(np.array(bounds[1:]), np.arange(T),
                                   side="right")
        cps = None
        for t in range(T):
            if cps is None:
                cps = cpool.tile([128, 8, 4, HID], f32, tag="cps",
                                 name=f"cps_b{t // 32}")
            ch = int(chunk_of[t])
            base = (t - bounds[ch]) * RW
            q, j8 = t % 4, (t // 4) % 8
            kk = 96 + int(K[4 * t + 3])
            lhsT = ch_sb[ch][0:kk, base:base + EFD]
            rhs = ch_sb[ch][0:kk, base + EFD:base + RW]
            out = cps[32 * q:32 * q + 32, j8, :, :]
            nc.tensor.matmul(out, lhsT, rhs, start=True, stop=True,
                             tile_position=(0, 32 * q))
            if t % 32 == 31 or t == T - 1:
                b = t // 32
                src = cps[:, :, :, :].transpose([0, 3, 1, 2])
                if b % 2 == 0:
                    nc.vector.tensor_copy(c_all[:, :, 32 * b:32 * b + 32], src)
                else:
                    nc.scalar.copy(c_all[:, :, 32 * b:32 * b + 32], src)
                cps = None
                if b == NBA - 1:
                    # early W pass over banks 0..NBA while the PE would
                    # otherwise stall waiting on late input chunks
                    w_stage(0, NBA)
        w_stage(NBA, NB)
    nc.compile()
    return nc


def _assemble(outs, segs_per_core, NPOS, NB):
    WND = NB * 32
    out = np.zeros((N_NODES, MSG), dtype=np.float32)
    j = np.arange(NPOS)
    t = j // 4
    g = j % 4
    q = t % 4
    j8 = (t // 4) % 8
    b = t // 32
    w2 = 4 * j8 + g
    col = 32 * b + w2
    for c in range(NCORES):
        out_sb = outs[c]
        pos_rows = np.empty((NPOS, MSG), dtype=np.float32)
        for half in range(2):
            pos_rows[:, 32 * half:32 * half + 32] = \
                out_sb[32 * q[:, None] + np.arange(32)[None, :],
                       (WND * half + col)[:, None]]
        segs = segs_per_core[c]
        for jj in range(min(len(segs), NPOS)):
            n, _ = segs[jj]
            out[n] += pos_rows[jj]
    return out


def kernel(node_features, edge_features, edge_sources, edge_targets,
           hidden, initial, W, b):
    from concourse.bass_utils import run_bass_kernel_spmd

    edge_targets = np.asarray(edge_targets)
    edge_sources = np.asarray(edge_sources)
    edge_features = np.asarray(edge_features, dtype=np.float32)
    hidden = np.asarray(hidden, dtype=np.float32)
    W = np.asarray(W, dtype=np.float32)
    b = np.asarray(b, dtype=np.float32)

    key = edge_targets.tobytes()
    if key in _CACHE:
        layout, nc = _CACHE[key]
    else:
        layout = _build_layout(edge_targets)
        segs_per_core, NPOS, K, T, NB = layout
        assert K.max() <= 32
        nc = _build_program(NPOS, K, T, NB)
        _CACHE[key] = (layout, nc)
    segs_per_core, NPOS, K, T, NB = layout

    wbd = _build_wrep(W)
    in_maps = []
    for c in range(NCORES):
        data = _pack_core(segs_per_core[c], NPOS, K, T, wbd,
                          edge_features, edge_sources, hidden)
        in_maps.append({"data": data})

    res = run_bass_kernel_spmd(nc, in_maps, list(range(NCORES)))
    outs = [res.results[c]["out"] for c in range(NCORES)]
    out = _assemble(outs, segs_per_core, NPOS, NB)

    if np.any(b):
        # bias term: out[n] += (sum_{e->n} hidden[src e]) @ Br,
        # Br[h, m] = b[m*16+h].  (b is all-zero for this problem.)
        Br = b.reshape(MSG, HID).T.astype(np.float32)
        acc = np.zeros((N_NODES, HID), dtype=np.float32)
        np.add.at(acc, edge_targets, hidden[edge_sources])
        out += acc @ Br
    return out


# revision 39
# speedup vs baseline: 1.1631x; 1.1631x over previous
"""Trainium2 Bass kernel for nn_MessageLayer (GNN message passing), 8 NeuronCores.

Reference computation:
    edge_mat = (edge_features @ W + b).reshape(E, 64, 16)
    messages = einsum('emh,eh->em', edge_mat, hidden[edge_sources])
    out      = segment_sum(messages, edge_targets, num_segments=10000)

Algebraic restructure (cuts FLOPs 32x): since aggregation is linear,
    out[n, m] = sum_{f,h} W[f, m*16+h] * C[n, f, h],
    C[n, f, h] = sum_{e: tgt(e)=n} ef[e, f] * hidden[src(e), h]
Then out = C @ Wr as 32 accumulating matmuls against a block-diagonal W.

C-stage structure (v5): the PE issue rate (~35 ns per matmul call, nearly
independent of operand size) dominates, so segments are batched FOUR per
matmul.  Segments are split to <=32 edges and packed four to a 128-row tile
(group g = rows 32g..32g+32).  Per tile ONE matmul:
    lhsT = ef   [rows, 32]   (compact edge features, all four groups)
    rhs  = nhbd [rows, 64]   (block-diagonal: group g's source-hidden in
                              cols 16g..16g+16, zeros elsewhere)
    out  = [32f, 64=(g,h)] in PSUM at partition group 32q, q = tile%4
so C for four segments lands in one PSUM write.  All matmuls contract from
row 0 (same PE row-group) so they serialize — no concurrent PSUM-bank
drains (the known wedge).  The block-diagonal rhs is packed on the host
(zeros ship from HBM; ~2x input bytes, still cheaper than 3x the matmul
calls).

All matmul operands are bf16 (PSUM stays fp32): 1 cycle/row vs fp32's 4,
half the DMA.  Quantization error ~0.3% vs the 2e-2 gate.

Sharding: node-ownership, no collective.  Nodes are dealt snake-wise in
descending-degree order so per-core sorted segment profiles match and the
SPMD cross-core max-padding (K_j) is minimal.

c_all is h-major so the W-stage moving operand c_all[:, h, :] is contiguous
(a strided moving was measured 3x slower).  Every input chunk is split
across both HW DGE queues (SP + Activation) so each chunk completes at full
aggregate HBM rate.
"""
import numpy as np
import ml_dtypes
from contextlib import ExitStack

BF16 = ml_dtypes.bfloat16

N_NODES = 10000
N_EDGES = 320000
HID = 16
MSG = 64
EFD = 32
NCORES = 8
RW = EFD + 4 * HID               # 96 packed cols per row: ef | nhbd(4x16)
# progressive input chunks: small first chunk so the PE starts early, taper
# at the end so the last casts (which gate the W stage) come early
CHUNK_FRACS = (0.0, 0.015, 0.05, 0.10, 0.16, 0.23, 0.31, 0.40, 0.49, 0.58,
               0.67, 0.76, 0.85, 0.93, 1.0)
NCHUNK = len(CHUNK_FRACS) - 1
WBD_AFTER = 3                    # ship wbd after this chunk (needed by W-A)
W_SPLITS = (6, 11)               # early W passes after these bank counts

_CACHE = {}


def _build_layout(edge_targets):
    """Per-core segment lists (<=32 edges each, K-sorted) + SPMD-uniform K."""
    deg = np.bincount(edge_targets, minlength=N_NODES)
    order = np.argsort(-deg, kind="stable")      # nodes by degree desc
    node_core = np.empty(N_NODES, dtype=np.int64)
    snake = list(range(NCORES)) + list(range(NCORES - 1, -1, -1))
    for i, n in enumerate(order):
        node_core[n] = snake[i % (2 * NCORES)]

    order_e = np.argsort(edge_targets, kind="stable")
    tgt_sorted = edge_targets[order_e]
    uniq, starts = np.unique(tgt_sorted, return_index=True)
    bounds = list(starts) + [len(tgt_sorted)]

    segs_per_core = [[] for _ in range(NCORES)]
    for i, n in enumerate(uniq):
        s, e = bounds[i], bounds[i + 1]
        c = node_core[n]
        while e - s > 32:           # split to <=32; host re-adds partials
            segs_per_core[c].append((int(n), order_e[s:s + 32]))
            s += 32
        segs_per_core[c].append((int(n), order_e[s:e]))
    for c in range(NCORES):
        segs_per_core[c].sort(key=lambda t: -len(t[1]))

    NPOS = max(len(s) for s in segs_per_core)
    NPOS = ((NPOS + 3) // 4) * 4
    K = np.ones(NPOS, dtype=np.int64)
    for segs in segs_per_core:
        for j, (_, e) in enumerate(segs):
            K[j] = max(K[j], len(e))
    T = NPOS // 4
    NB = (T + 31) // 32
    return segs_per_core, NPOS, K, T, NB


def _pack_core(segs, NPOS, K, T, wbd, edge_features, edge_sources, hidden):
    # position j -> tile t=j//4, group g=j%4, rows 32g..32g+K_j of tile t
    ef = np.zeros((T * 128, EFD), dtype=np.float32)
    nh = np.zeros((T * 128, HID), dtype=np.float32)
    for j in range(min(len(segs), NPOS)):
        _, eids = segs[j]
        base = (j // 4) * 128 + 32 * (j % 4)
        ef[base:base + len(eids)] = edge_features[eids]
        nh[base:base + len(eids)] = hidden[edge_sources[eids]]
    d = np.zeros((T * 128, RW), dtype=np.float32)
    d[:, :EFD] = ef
    g_of_row = (np.arange(T * 128) % 128) // 32
    for g in range(4):
        m = g_of_row == g
        d[m, EFD + HID * g:EFD + HID * (g + 1)] = nh[m]
    # DRAM [128 partitions, T*96 + wbd]: tile t at free offset 96t
    d = d.reshape(T, 128, RW).swapaxes(0, 1).reshape(128, T * RW)
    return np.ascontiguousarray(np.concatenate([d.astype(BF16), wbd], axis=1))


def _build_wrep(W):
    # Wbd[p=2h+half] [(q,f)=128, (q',mh)=128] = delta_qq' W[f, (mh+32*half)*16+h]
    wbd = np.zeros((32, 128, 128), dtype=np.float32)
    Wr = W.reshape(EFD, MSG, HID)                      # [f, m, h]
    for h in range(HID):
        for half in range(2):
            p = 2 * h + half
            blk = Wr[:, 32 * half:32 * half + 32, h]   # [f=32, mh=32]
            for q in range(4):
                wbd[p, 32 * q:32 * q + 32, 32 * q:32 * q + 32] = blk
    # DRAM layout [128, 32*128]: phase p at free offset 128p
    return np.ascontiguousarray(
        wbd.transpose(1, 0, 2).reshape(128, 32 * 128)).astype(BF16)


def _chunk_bounds(T):
    b = [round(f * T) for f in CHUNK_FRACS]
    b[-1] = T
    return b


def _build_program(NPOS, K, T, NB):
    import concourse.tile as tile
    from concourse import bacc, mybir

    f32 = mybir.dt.float32
    bf16 = mybir.dt.bfloat16
    bounds = _chunk_bounds(T)

    nc = bacc.Bacc("TRN2", target_bir_lowering=False, debug=False,
                   num_devices=NCORES)
    data_dram = nc.dram_tensor("data", [128, T * RW + 32 * 128], bf16,
                               kind="ExternalInput").ap()
    out_dram = nc.dram_tensor("out", [128, 2 * NB * 32], f32,
                              kind="ExternalOutput").ap()

    with tile.TileContext(nc) as tc, ExitStack() as ctx:
        big = ctx.enter_context(tc.tile_pool(name="big", bufs=1))
        cpool = ctx.enter_context(tc.tile_pool(name="cps", bufs=3,
                                               space="PSUM"))
        opool = ctx.enter_context(tc.tile_pool(name="ops", bufs=2, space="PSUM"))

        ch_sb = []
        wbd_sb = None
        for k in range(NCHUNK):
            lo, hi = bounds[k] * RW, bounds[k + 1] * RW
            t = big.tile([128, hi - lo], bf16, tag=f"ch{k}", name=f"ch{k}")
            # full-width transfers, alternating HW DGE queues (a [64,:]
            # partition split measured only ~275 GB/s aggregate vs ~405)
            eng = nc.sync if k % 2 == 0 else nc.scalar
            eng.dma_start(t[:], data_dram[:, lo:hi])
            ch_sb.append(t)
            if k == WBD_AFTER:   # wbd mid-stream, split across both queues
                wbd_sb = big.tile([128, 32 * 128], bf16, tag="wbd",
                                  name="wbd")
                nc.sync.dma_start(wbd_sb[:, :2048],
                                  data_dram[:, T * RW:T * RW + 2048])
                nc.scalar.dma_start(wbd_sb[:, 2048:],
                                    data_dram[:, T * RW + 2048:])

        # h-major: c_all[p, h, 32*b + w2], w2 = 4*j8 + g.  Columns of the
        # partial last bank with no backing tile carry PSUM garbage; the
        # W-stage is column-independent and _assemble never reads them.
        c_all = big.tile([128, HID, NB * 32], bf16, tag="call")

        out_sb = big.tile([128, 2 * NB * 32], f32, tag="outsb")

        def w_stage(lo_b, hi_b):
            # out = C @ Wr restricted to banks [lo_b, hi_b).
            # po is a full PSUM bank: two sub-bank po tiles can share a
            # bank, and a start=True reset clobbers the bank-mate.
            lo, hi = 32 * lo_b, 32 * hi_b
            for half in range(2):
                po = opool.tile([128, 512], f32, tag="po",
                                name=f"po{lo_b}_{half}")[:, :hi - lo]
                for h in range(HID):
                    p = 2 * h + half
                    nc.tensor.matmul(
                        po[:], wbd_sb[:, 128 * p:128 * p + 128],
                        c_all[:, h, lo:hi],
                        start=(h == 0), stop=(h == HID - 1))
                sl = slice(NB * 32 * half + lo, NB * 32 * half + hi)
                if half == 0:
                    nc.vector.tensor_copy(out_sb[:, sl], po[:])
                else:
                    nc.scalar.copy(out_sb[:, sl], po[:])
                (nc.sync if half == 0 else nc.scalar).dma_start(
                    out_dram[:, sl], out_sb[:, sl])

        chunk_of = np.searchsorted(np.array(bounds[1:]), np.arange(T),
                                   side="right")
        cps = None
        for t in range(T):
            if cps is None:
                cps = cpool.tile([128, 8, 4, HID], f32, tag="cps",
                                 name=f"cps_b{t // 32}")
            ch = int(chunk_of[t])
            base = (t - bounds[ch]) * RW
            q, j8 = t % 4, (t // 4) % 8
            kk = 96 + int(K[4 * t + 3])
            lhsT = ch_sb[ch][0:kk, base:base + EFD]
            rhs = ch_sb[ch][0:kk, base + EFD:base + RW]
            out = cps[32 * q:32 * q + 32, j8, :, :]
            nc.tensor.matmul(out, lhsT, rhs, start=True, stop=True,
                             tile_position=(0, 32 * q))
            if t % 32 == 31 or t == T - 1:
                b = t // 32
                src = cps[:, :, :, :].transpose([0, 3, 1, 2])
                if b % 2 == 0:
                    nc.vector.tensor_copy(c_all[:, :, 32 * b:32 * b + 32], src)
                else:
                    nc.scalar.copy(c_all[:, :, 32 * b:32 * b + 32], src)
                cps = None
                # early W passes while the PE would otherwise stall
                # waiting on late input chunks
                if b + 1 in W_SPLITS:
                    i = W_SPLITS.index(b + 1)
                    w_stage(W_SPLITS[i - 1] if i else 0, b + 1)
        w_stage(W_SPLITS[-1], NB)
    nc.compile()
    return nc


def _assemble(outs, segs_per_core, NPOS, NB):
    WND = NB * 32
    out = np.zeros((N_NODES, MSG), dtype=np.float32)
    j = np.arange(NPOS)
    t = j // 4
    g = j % 4
    q = t % 4
    j8 = (t // 4) % 8
    b = t // 32
    w2 = 4 * j8 + g
    col = 32 * b + w2
    for c in range(NCORES):
        out_sb = outs[c]
        pos_rows = np.empty((NPOS, MSG), dtype=np.float32)
        for half in range(2):
            pos_rows[:, 32 * half:32 * half + 32] = \
                out_sb[32 * q[:, None] + np.arange(32)[None, :],
                       (WND * half + col)[:, None]]
        segs = segs_per_core[c]
        for jj in range(min(len(segs), NPOS)):
            n, _ = segs[jj]
            out[n] += pos_rows[jj]
    return out


def kernel(node_features, edge_features, edge_sources, edge_targets,
           hidden, initial, W, b):
    from concourse.bass_utils import run_bass_kernel_spmd

    edge_targets = np.asarray(edge_targets)
    edge_sources = np.asarray(edge_sources)
    edge_features = np.asarray(edge_features, dtype=np.float32)
    hidden = np.asarray(hidden, dtype=np.float32)
    W = np.asarray(W, dtype=np.float32)
    b = np.asarray(b, dtype=np.float32)

    key = edge_targets.tobytes()
    if key in _CACHE:
        layout, nc = _CACHE[key]
    else:
        layout = _build_layout(edge_targets)
        segs_per_core, NPOS, K, T, NB = layout
        assert K.max() <= 32
        nc = _build_program(NPOS, K, T, NB)
        _CACHE[key] = (layout, nc)
    segs_per_core, NPOS, K, T, NB = layout

    wbd = _build_wrep(W)
    in_maps = []
    for c in range(NCORES):
        data = _pack_core(segs_per_core[c], NPOS, K, T, wbd,
                          edge_features, edge_sources, hidden)
        in_maps.append({"data": data})

    res = run_bass_kernel_spmd(nc, in_maps, list(range(NCORES)))
    outs = [res.results[c]["out"] for c in range(NCORES)]
    out = _assemble(outs, segs_per_core, NPOS, NB)

    if np.any(b):
        # bias term: out[n] += (sum_{e->n} hidden[src e]) @ Br,
        # Br[h, m] = b[m*16+h].  (b is all-zero for this problem.)
        Br = b.reshape(MSG, HID).T.astype(np.float32)
        acc = np.zeros((N_NODES, HID), dtype=np.float32)
        np.add.at(acc, edge_targets, hidden[edge_sources])
        out += acc @ Br
    return out


# revision 40
# speedup vs baseline: 1.1788x; 1.0135x over previous
"""Trainium2 Bass kernel for nn_MessageLayer (GNN message passing), 8 NeuronCores.

Reference computation:
    edge_mat = (edge_features @ W + b).reshape(E, 64, 16)
    messages = einsum('emh,eh->em', edge_mat, hidden[edge_sources])
    out      = segment_sum(messages, edge_targets, num_segments=10000)

Algebraic restructure (cuts FLOPs 32x): since aggregation is linear,
    out[n, m] = sum_{f,h} W[f, m*16+h] * C[n, f, h],
    C[n, f, h] = sum_{e: tgt(e)=n} ef[e, f] * hidden[src(e), h]
Then out = C @ Wr as 32 accumulating matmuls against a block-diagonal W.

C-stage structure (v5): the PE issue rate (~35 ns per matmul call, nearly
independent of operand size) dominates, so segments are batched FOUR per
matmul.  Segments are split to <=32 edges and packed four to a 128-row tile
(group g = rows 32g..32g+32).  Per tile ONE matmul:
    lhsT = ef   [rows, 32]   (compact edge features, all four groups)
    rhs  = nhbd [rows, 64]   (block-diagonal: group g's source-hidden in
                              cols 16g..16g+16, zeros elsewhere)
    out  = [32f, 64=(g,h)] in PSUM at partition group 32q, q = tile%4
so C for four segments lands in one PSUM write.  All matmuls contract from
row 0 (same PE row-group) so they serialize — no concurrent PSUM-bank
drains (the known wedge).  The block-diagonal rhs is packed on the host
(zeros ship from HBM; ~2x input bytes, still cheaper than 3x the matmul
calls).

All matmul operands are bf16 (PSUM stays fp32): 1 cycle/row vs fp32's 4,
half the DMA.  Quantization error ~0.3% vs the 2e-2 gate.

Sharding: node-ownership, no collective.  Nodes are dealt snake-wise in
descending-degree order so per-core sorted segment profiles match and the
SPMD cross-core max-padding (K_j) is minimal.

c_all is h-major so the W-stage moving operand c_all[:, h, :] is contiguous
(a strided moving was measured 3x slower).  Every input chunk is split
across both HW DGE queues (SP + Activation) so each chunk completes at full
aggregate HBM rate.
"""
import numpy as np
import ml_dtypes
from contextlib import ExitStack

BF16 = ml_dtypes.bfloat16

N_NODES = 10000
N_EDGES = 320000
HID = 16
MSG = 64
EFD = 32
NCORES = 8
RW = EFD + 4 * HID               # 96 packed cols per row: ef | nhbd(4x16)
# progressive input chunks: small first chunk so the PE starts early, taper
# at the end so the last casts (which gate the W stage) come early
CHUNK_FRACS = (0.0, 0.015, 0.05, 0.10, 0.16, 0.23, 0.31, 0.40, 0.49, 0.58,
               0.67, 0.76, 0.85, 0.93, 1.0)
NCHUNK = len(CHUNK_FRACS) - 1
WBD_AFTER = 1                    # ship wbd after this chunk (needed by W-A)
W_SPLITS = (3, 6, 9, 12)         # early W passes after these bank counts

_CACHE = {}


def _build_layout(edge_targets):
    """Per-core segment lists (<=32 edges each, K-sorted) + SPMD-uniform K."""
    deg = np.bincount(edge_targets, minlength=N_NODES)
    order = np.argsort(-deg, kind="stable")      # nodes by degree desc
    node_core = np.empty(N_NODES, dtype=np.int64)
    snake = list(range(NCORES)) + list(range(NCORES - 1, -1, -1))
    for i, n in enumerate(order):
        node_core[n] = snake[i % (2 * NCORES)]

    order_e = np.argsort(edge_targets, kind="stable")
    tgt_sorted = edge_targets[order_e]
    uniq, starts = np.unique(tgt_sorted, return_index=True)
    bounds = list(starts) + [len(tgt_sorted)]

    segs_per_core = [[] for _ in range(NCORES)]
    for i, n in enumerate(uniq):
        s, e = bounds[i], bounds[i + 1]
        c = node_core[n]
        while e - s > 32:           # split to <=32; host re-adds partials
            segs_per_core[c].append((int(n), order_e[s:s + 32]))
            s += 32
        segs_per_core[c].append((int(n), order_e[s:e]))
    for c in range(NCORES):
        segs_per_core[c].sort(key=lambda t: -len(t[1]))

    NPOS = max(len(s) for s in segs_per_core)
    NPOS = ((NPOS + 3) // 4) * 4
    K = np.ones(NPOS, dtype=np.int64)
    for segs in segs_per_core:
        for j, (_, e) in enumerate(segs):
            K[j] = max(K[j], len(e))
    T = NPOS // 4
    NB = (T + 31) // 32
    return segs_per_core, NPOS, K, T, NB


def _pack_core(segs, NPOS, K, T, wbd, edge_features, edge_sources, hidden):
    # position j -> tile t=j//4, group g=j%4, rows 32g..32g+K_j of tile t
    ef = np.zeros((T * 128, EFD), dtype=np.float32)
    nh = np.zeros((T * 128, HID), dtype=np.float32)
    for j in range(min(len(segs), NPOS)):
        _, eids = segs[j]
        base = (j // 4) * 128 + 32 * (j % 4)
        ef[base:base + len(eids)] = edge_features[eids]
        nh[base:base + len(eids)] = hidden[edge_sources[eids]]
    d = np.zeros((T * 128, RW), dtype=np.float32)
    d[:, :EFD] = ef
    g_of_row = (np.arange(T * 128) % 128) // 32
    for g in range(4):
        m = g_of_row == g
        d[m, EFD + HID * g:EFD + HID * (g + 1)] = nh[m]
    # DRAM [128 partitions, T*96 + wbd]: tile t at free offset 96t
    d = d.reshape(T, 128, RW).swapaxes(0, 1).reshape(128, T * RW)
    return np.ascontiguousarray(np.concatenate([d.astype(BF16), wbd], axis=1))


def _build_wrep(W):
    # Wbd[p=2h+half] [(q,f)=128, (q',mh)=128] = delta_qq' W[f, (mh+32*half)*16+h]
    wbd = np.zeros((32, 128, 128), dtype=np.float32)
    Wr = W.reshape(EFD, MSG, HID)                      # [f, m, h]
    for h in range(HID):
        for half in range(2):
            p = 2 * h + half
            blk = Wr[:, 32 * half:32 * half + 32, h]   # [f=32, mh=32]
            for q in range(4):
                wbd[p, 32 * q:32 * q + 32, 32 * q:32 * q + 32] = blk
    # DRAM layout [128, 32*128]: phase p at free offset 128p
    return np.ascontiguousarray(
        wbd.transpose(1, 0, 2).reshape(128, 32 * 128)).astype(BF16)


def _chunk_bounds(T):
    b = [round(f * T) for f in CHUNK_FRACS]
    b[-1] = T
    return b


def _build_program(NPOS, K, T, NB):
    import concourse.tile as tile
    from concourse import bacc, mybir

    f32 = mybir.dt.float32
    bf16 = mybir.dt.bfloat16
    bounds = _chunk_bounds(T)

    nc = bacc.Bacc("TRN2", target_bir_lowering=False, debug=False,
                   num_devices=NCORES)
    data_dram = nc.dram_tensor("data", [128, T * RW + 32 * 128], bf16,
                               kind="ExternalInput").ap()
    out_dram = nc.dram_tensor("out", [128, 2 * NB * 32], f32,
                              kind="ExternalOutput").ap()

    with tile.TileContext(nc) as tc, ExitStack() as ctx:
        big = ctx.enter_context(tc.tile_pool(name="big", bufs=1))
        cpool = ctx.enter_context(tc.tile_pool(name="cps", bufs=3,
                                               space="PSUM"))
        opool = ctx.enter_context(tc.tile_pool(name="ops", bufs=2, space="PSUM"))

        ch_sb = []
        wbd_sb = None
        for k in range(NCHUNK):
            lo, hi = bounds[k] * RW, bounds[k + 1] * RW
            t = big.tile([128, hi - lo], bf16, tag=f"ch{k}", name=f"ch{k}")
            # full-width transfers, alternating HW DGE queues (a [64,:]
            # partition split measured only ~275 GB/s aggregate vs ~405)
            eng = nc.sync if k % 2 == 0 else nc.scalar
            eng.dma_start(t[:], data_dram[:, lo:hi])
            ch_sb.append(t)
            if k == WBD_AFTER:   # wbd mid-stream, split across both queues
                wbd_sb = big.tile([128, 32 * 128], bf16, tag="wbd",
                                  name="wbd")
                nc.sync.dma_start(wbd_sb[:, :2048],
                                  data_dram[:, T * RW:T * RW + 2048])
                nc.scalar.dma_start(wbd_sb[:, 2048:],
                                    data_dram[:, T * RW + 2048:])

        # h-major: c_all[p, h, 32*b + w2], w2 = 4*j8 + g.  Columns of the
        # partial last bank with no backing tile carry PSUM garbage; the
        # W-stage is column-independent and _assemble never reads them.
        c_all = big.tile([128, HID, NB * 32], bf16, tag="call")

        out_sb = big.tile([128, 2 * NB * 32], f32, tag="outsb")

        def w_stage(lo_b, hi_b):
            # out = C @ Wr restricted to banks [lo_b, hi_b).
            # po is a full PSUM bank: two sub-bank po tiles can share a
            # bank, and a start=True reset clobbers the bank-mate.
            lo, hi = 32 * lo_b, 32 * hi_b
            for half in range(2):
                po = opool.tile([128, 512], f32, tag="po",
                                name=f"po{lo_b}_{half}")[:, :hi - lo]
                for h in range(HID):
                    p = 2 * h + half
                    nc.tensor.matmul(
                        po[:], wbd_sb[:, 128 * p:128 * p + 128],
                        c_all[:, h, lo:hi],
                        start=(h == 0), stop=(h == HID - 1))
                sl = slice(NB * 32 * half + lo, NB * 32 * half + hi)
                if half == 0:
                    nc.vector.tensor_copy(out_sb[:, sl], po[:])
                else:
                    nc.scalar.copy(out_sb[:, sl], po[:])
                (nc.sync if half == 0 else nc.scalar).dma_start(
                    out_dram[:, sl], out_sb[:, sl])

        chunk_of = np.searchsorted(np.array(bounds[1:]), np.arange(T),
                                   side="right")
        cps = None
        for t in range(T):
            if cps is None:
                cps = cpool.tile([128, 8, 4, HID], f32, tag="cps",
                                 name=f"cps_b{t // 32}")
            ch = int(chunk_of[t])
            base = (t - bounds[ch]) * RW
            q, j8 = t % 4, (t // 4) % 8
            kk = 96 + int(K[4 * t + 3])
            lhsT = ch_sb[ch][0:kk, base:base + EFD]
            rhs = ch_sb[ch][0:kk, base + EFD:base + RW]
            out = cps[32 * q:32 * q + 32, j8, :, :]
            nc.tensor.matmul(out, lhsT, rhs, start=True, stop=True,
                             tile_position=(0, 32 * q))
            if t % 32 == 31 or t == T - 1:
                b = t // 32
                src = cps[:, :, :, :].transpose([0, 3, 1, 2])
                if b % 2 == 0:
                    nc.vector.tensor_copy(c_all[:, :, 32 * b:32 * b + 32], src)
                else:
                    nc.scalar.copy(c_all[:, :, 32 * b:32 * b + 32], src)
                cps = None
                # early W passes while the PE would otherwise stall
                # waiting on late input chunks
                if b + 1 in W_SPLITS:
                    i = W_SPLITS.index(b + 1)
                    w_stage(W_SPLITS[i - 1] if i else 0, b + 1)
        w_stage(W_SPLITS[-1], NB)
    nc.compile()
    return nc


def _assemble(outs, segs_per_core, NPOS, NB):
    WND = NB * 32
    out = np.zeros((N_NODES, MSG), dtype=np.float32)
    j = np.arange(NPOS)
    t = j // 4
    g = j % 4
    q = t % 4
    j8 = (t // 4) % 8
    b = t // 32
    w2 = 4 * j8 + g
    col = 32 * b + w2
    for c in range(NCORES):
        out_sb = outs[c]
        pos_rows = np.empty((NPOS, MSG), dtype=np.float32)
        for half in range(2):
            pos_rows[:, 32 * half:32 * half + 32] = \
                out_sb[32 * q[:, None] + np.arange(32)[None, :],
                       (WND * half + col)[:, None]]
        segs = segs_per_core[c]
        for jj in range(min(len(segs), NPOS)):
            n, _ = segs[jj]
            out[n] += pos_rows[jj]
    return out


def kernel(node_features, edge_features, edge_sources, edge_targets,
           hidden, initial, W, b):
    from concourse.bass_utils import run_bass_kernel_spmd

    edge_targets = np.asarray(edge_targets)
    edge_sources = np.asarray(edge_sources)
    edge_features = np.asarray(edge_features, dtype=np.float32)
    hidden = np.asarray(hidden, dtype=np.float32)
    W = np.asarray(W, dtype=np.float32)
    b = np.asarray(b, dtype=np.float32)

    key = edge_targets.tobytes()
    if key in _CACHE:
        layout, nc = _CACHE[key]
    else:
        layout = _build_layout(edge_targets)
        segs_per_core, NPOS, K, T, NB = layout
        assert K.max() <= 32
        nc = _build_program(NPOS, K, T, NB)
        _CACHE[key] = (layout, nc)
    segs_per_core, NPOS, K, T, NB = layout

    wbd = _build_wrep(W)
    in_maps = []
    for c in range(NCORES):
        data = _pack_core(segs_per_core[c], NPOS, K, T, wbd,
                          edge_features, edge_sources, hidden)
        in_maps.append({"data": data})

    res = run_bass_kernel_spmd(nc, in_maps, list(range(NCORES)))
    outs = [res.results[c]["out"] for c in range(NCORES)]
    out = _assemble(outs, segs_per_core, NPOS, NB)

    if np.any(b):
        # bias term: out[n] += (sum_{e->n} hidden[src e]) @ Br,
        # Br[h, m] = b[m*16+h].  (b is all-zero for this problem.)
        Br = b.reshape(MSG, HID).T.astype(np.float32)
        acc = np.zeros((N_NODES, HID), dtype=np.float32)
        np.add.at(acc, edge_targets, hidden[edge_sources])
        out += acc @ Br
    return out
